# revision 2
# baseline (speedup 1.0000x reference)
"""Trainium2 Bass kernel for unscaled Luong dot-product attention.

Problem: B=16, Tq=Tk=D=1024, fp32.
    scores = Q @ E^T ; weights = softmax(scores, -1) ; out = weights @ E

Sharding: data-parallel over batch — each of the 8 NeuronCores processes
2 batches end-to-end; no cross-core communication.

Layout strategy: the host-side sharding step (inside kernel()) rearranges
each core's inputs so no on-device transposition of Q or E is needed:
  - q is shipped per q-block as [qb, d-part, dc, j] (i.e. Q^T tiled), so
    each 128-row q-block's stationary operands DMA straight into SBUF.
  - e is shipped twice: natural [k-part, kc, d] (bmm2 rhs) and transposed
    [d-part, dc, k] (bmm1 rhs). One 2 MB DMA each per batch.
q/et are shipped as fp16: the 11-bit mantissa keeps the softmax-amplified
score error ~6e-3. fp8 variants measured over the 2e-2 gate (bmm2 with
e4m3 W and E alone gives 2.7e-2 — softmax rows here are near-one-hot and
near-tie weights quantize at ~3%), so both bmms stay on the 1-cycle/row
16-bit path. The output is stored fp16 and widened to fp32 on the host.
Measured rel_l2 = 2.289e-3 vs the fp32 reference (gate 2e-2).

The PE matmul stream is the wall: walrus emits one LDWEIGHTS per matmul
(512 per rep; --enable-ldw-opt is hardcoded off and crashes walrus when
forced on), but an mm-only ablation measures ~110-115 us/core ~= the
512 x 213 ns pure-streaming roofline, i.e. LDWEIGHTS is effectively
hidden by the PE's load-ahead. So this revision removes everything else
from the PE stream and every stall around it:
  - W^T for bmm2 goes through the DMA crossbar transpose (16x128 xbar
    tiles, ~0.9 us/block on the SDMA engines) instead of PE
    transpose-mode — no transposes or identity (re)loads on the PE.
    (N=1024 moving operands would halve instruction count but are
    rejected by the ISA: fp32 PSUM output would span 2 banks.)
  - The depth-3 software pipeline is FLAT across blocks, batches, and
    in-NEFF reps: PE order is bmm1(i+2), bmm2(i) over the whole
    sequence, so each block's max->exp->xbar-transpose chain (~4 us)
    has two bmm1 + two bmm2 slots (~14 us) of cover and the pipeline
    never drains at batch boundaries.
PSUM: 2x scores + 2x ctx [128,1024] fp32 tiles = all 8 banks.

Per-block stages:
  front: DMA q-block tiles, bmm1 into PSUM dc-outer.
  mid:   negated row-max (DVE) -> exp with per-partition bias and fused
         row-sum (ACT, bf16 out) -> reciprocal (DVE) -> W^T via
         dma_start_transpose (sync/HWDGE queue).
  back:  bmm2 kc-outer -> fold 1/rowsum into the PSUM->SBUF copy (DVE,
         fp16 out) -> DMA out.

Measured (paired min-estimator, R=32 differencing): ~119 us/core vs
~131 us/core for the PE-transpose per-batch-pipeline predecessor, with
the mm-only ablation at ~115 us in the same session.
"""

import numpy as np

import concourse.bass as bass
import concourse.tile as tile
from concourse import bacc, mybir

P = 128
B_PER_CORE = 2
T = 1024  # Tq = Tk
D = 1024
NC_CHUNKS = T // P  # 8 k-chunks / q-blocks
ND_CHUNKS = D // P  # 8 d-chunks
F32 = mybir.dt.float32
BF16 = mybir.dt.bfloat16
F16 = mybir.dt.float16


def build_nc(reps: int = 1, mm_only: bool = False, salt: float | None = None):
    nc = bacc.Bacc("TRN2", target_bir_lowering=False, debug=False)
    # q: [b, qb, p, dc*128+j] = Q[b, qb*128+j, dc*128+p]  (Q^T, block-tiled)
    q_dram = nc.dram_tensor(
        "q", [B_PER_CORE, NC_CHUNKS, P, D], F16, kind="ExternalInput"
    ).ap()
    # e: [b, p, kc*1024+d] = E[b, kc*128+p, d]  (natural, partition-tiled)
    e_dram = nc.dram_tensor(
        "e", [B_PER_CORE, P, NC_CHUNKS * D], BF16, kind="ExternalInput"
    ).ap()
    # et: [b, p, dc*1024+k] = E[b, k, dc*128+p]  (transposed, partition-tiled)
    et_dram = nc.dram_tensor(
        "et", [B_PER_CORE, P, ND_CHUNKS * T], F16, kind="ExternalInput"
    ).ap()
    o_dram = nc.dram_tensor("o", [B_PER_CORE, T, D], F16, kind="ExternalOutput").ap()

    with tile.TileContext(nc) as tc:
        with (
            tc.tile_pool(name="const", bufs=1) as const_pool,
            tc.tile_pool(name="e_r", bufs=2) as e_r_pool,
            tc.tile_pool(name="etr", bufs=2) as etr_pool,
            tc.tile_pool(name="qt", bufs=3) as qt_pool,
            tc.tile_pool(name="w", bufs=3) as w_pool,
            tc.tile_pool(name="wt", bufs=3) as wt_pool,
            tc.tile_pool(name="ctx", bufs=2) as ctx_pool,
            tc.tile_pool(name="stat", bufs=4) as stat_pool,
            tc.tile_pool(name="sc_ps", bufs=2, space="PSUM") as sc_psum,
            tc.tile_pool(name="ctx_ps", bufs=2, space="PSUM") as ctx_psum,
        ):
            wt_const = None
            if mm_only:
                wt_const = const_pool.tile([P, NC_CHUNKS, P], BF16)
                nc.vector.memset(wt_const[:], 0.001)
            if salt is not None:
                # distinct BIR/HLO so compile caches can't cross-contaminate
                # otherwise-identical variants
                salt_t = const_pool.tile([P, 1], F32)
                nc.vector.memset(salt_t[:], salt)

            # flat block sequence across reps and batches
            batches_seq = [b for _ in range(reps) for b in range(B_PER_CORE)]
            blocks = [
                (bi, qb) for bi in range(len(batches_seq)) for qb in range(NC_CHUNKS)
            ]
            n_blocks = len(blocks)
            e_tiles = {}

            def ensure_batch(bi):
                if bi in e_tiles or bi >= len(batches_seq):
                    return
                b = batches_seq[bi]
                e_r = e_r_pool.tile([P, NC_CHUNKS, D], BF16, name="e_r")
                nc.gpsimd.dma_start(e_r[:], e_dram[b])
                etr = etr_pool.tile([P, ND_CHUNKS, T], F16, name="etr")
                nc.gpsimd.dma_start(etr[:], et_dram[b])
                e_tiles[bi] = (e_r, etr)

            def emit_front(i):
                """DMA Q^T block, bmm1 into PSUM (dc outer: matmul pairs
                share the stationary operand)."""
                bi, qb = blocks[i]
                ensure_batch(bi)
                etr = e_tiles[bi][1]
                b = batches_seq[bi]
                qt = qt_pool.tile([P, ND_CHUNKS, P], F16, name="qt")
                nc.sync.dma_start(qt[:], q_dram[b, qb])
                sc_ps = sc_psum.tile([P, T], F32, name="sc_ps")
                for dc in range(ND_CHUNKS):
                    for kh in range(2):
                        nc.tensor.matmul(
                            sc_ps[:, kh * 512 : (kh + 1) * 512],
                            qt[:, dc, :],
                            etr[:, dc, kh * 512 : (kh + 1) * 512],
                            start=(dc == 0),
                            stop=(dc == ND_CHUNKS - 1),
                        )
                return sc_ps

            def emit_mid(i, sc_ps):
                """Softmax the block's scores; W^T via the DMA crossbar."""
                if mm_only:
                    return None
                negmax = stat_pool.tile([P, 1], F32, tag="negmax", name="negmax")
                nc.vector.tensor_reduce(
                    out=negmax[:],
                    in_=sc_ps[:],
                    op=mybir.AluOpType.max,
                    axis=mybir.AxisListType.X,
                    negate=True,
                )
                w_sb = w_pool.tile([P, T], BF16, name="w_sb")
                ssum = stat_pool.tile([P, 1], F32, tag="ssum", name="ssum")
                nc.scalar.activation(
                    w_sb[:],
                    sc_ps[:],
                    mybir.ActivationFunctionType.Exp,
                    bias=negmax[:],
                    accum_out=ssum[:],
                )
                recip = stat_pool.tile([P, 1], F32, tag="recip", name="recip")
                nc.vector.reciprocal(recip[:], ssum[:])
                wt = wt_pool.tile([P, NC_CHUNKS, P], BF16, name="wt")
                nc.sync.dma_start_transpose(wt[:], w_sb[:])
                return wt, recip

            def emit_back(i, mid, sc_ps=None):
                """bmm2 kc-outer; fold 1/rowsum into the PSUM->SBUF copy;
                DMA out."""
                bi, qb = blocks[i]
                e_r = e_tiles[bi][0]
                b = batches_seq[bi]
                if mm_only:
                    wt, recip = wt_const, None
                else:
                    wt, recip = mid
                ctx_ps = ctx_psum.tile([P, T], F32, name="ctx_ps")
                for kc in range(NC_CHUNKS):
                    for dh in range(2):
                        nc.tensor.matmul(
                            ctx_ps[:, dh * 512 : (dh + 1) * 512],
                            wt[:, kc, :],
                            e_r[:, kc, dh * 512 : (dh + 1) * 512],
                            start=(kc == 0),
                            stop=(kc == NC_CHUNKS - 1),
                        )
                ctx_sb = ctx_pool.tile([P, D], F16, name="ctx_sb")
                if mm_only:
                    nc.vector.tensor_copy(ctx_sb[:], ctx_ps[:])
                    nc.vector.tensor_copy(ctx_sb[:, 0:1], sc_ps[:, 0:1])
                else:
                    nc.vector.tensor_scalar_mul(ctx_sb[:], ctx_ps[:], recip[:])
                nc.sync.dma_start(o_dram[b, qb * P : (qb + 1) * P, :], ctx_sb[:])

            # depth-3 pipeline over the flat block list; PE order:
            # f0 f1 f2 b0 f3 b1 ...
            fronts = {0: emit_front(0), 1: emit_front(1)}
            mids = {0: emit_mid(0, fronts[0])}
            for i in range(n_blocks):
                if i + 2 < n_blocks:
                    fronts[i + 2] = emit_front(i + 2)
                if i + 1 < n_blocks:
                    mids[i + 1] = emit_mid(i + 1, fronts[i + 1])
                emit_back(i, mids.get(i), sc_ps=fronts[i])
                fronts.pop(i, None)
                mids.pop(i, None)

    nc.compile()
    return nc


def make_in_maps(decoder_hidden: np.ndarray, encoder_outputs: np.ndarray):
    """Host-side sharding + layout prep: per-core input dicts matching the
    DRAM tensor layouts declared in build_nc."""
    dh = np.asarray(decoder_hidden, dtype=np.float32)
    eo = np.asarray(encoder_outputs, dtype=np.float32)
    assert dh.shape == (16, T, D) and eo.shape == (16, T, D)
    import ml_dtypes

    in_maps = []
    for i in range(8):
        qc = dh[i * B_PER_CORE : (i + 1) * B_PER_CORE]
        ec = eo[i * B_PER_CORE : (i + 1) * B_PER_CORE]
        # [b, qb, j, dc, p] -> [b, qb, p, dc, j], fp16 (bmm1 stationary)
        qh = np.ascontiguousarray(
            qc.reshape(B_PER_CORE, NC_CHUNKS, P, ND_CHUNKS, P)
            .transpose(0, 1, 4, 3, 2)
            .astype(np.float16)
        ).reshape(B_PER_CORE, NC_CHUNKS, P, D)
        # [b, kc, p, d] -> [b, p, kc, d], bf16 for bmm2's rhs
        eh = np.ascontiguousarray(
            ec.reshape(B_PER_CORE, NC_CHUNKS, P, D)
            .transpose(0, 2, 1, 3)
            .astype(ml_dtypes.bfloat16)
        ).reshape(B_PER_CORE, P, NC_CHUNKS * D)
        # [b, k, dc, p] -> [b, p, dc, k], fp16 (bmm1 moving operand)
        eth = np.ascontiguousarray(
            ec.reshape(B_PER_CORE, T, ND_CHUNKS, P)
            .transpose(0, 3, 2, 1)
            .astype(np.float16)
        ).reshape(B_PER_CORE, P, ND_CHUNKS * T)
        in_maps.append({"q": qh, "e": eh, "et": eth})
    return in_maps


_NC_CACHE = None


def _get_nc():
    global _NC_CACHE
    if _NC_CACHE is None:
        _NC_CACHE = build_nc()
    return _NC_CACHE


def kernel(decoder_hidden: np.ndarray, encoder_outputs: np.ndarray) -> np.ndarray:
    import os

    # The axon client here has no NTFF profiling hook; make sure a stray
    # BASS_TRACE in the environment can't push run_bass_kernel_spmd onto
    # the tracing path.
    os.environ["BASS_NEVER_TRACE"] = "1"
    from concourse import bass_utils

    nc = _get_nc()
    in_maps = make_in_maps(decoder_hidden, encoder_outputs)
    res = bass_utils.run_bass_kernel_spmd(nc, in_maps, core_ids=list(range(8)))
    return np.concatenate([r["o"] for r in res.results], axis=0).astype(np.float32)


# revision 4
# speedup vs baseline: 1.9279x; 1.9279x over previous
"""Trainium2 Bass kernel for unscaled Luong dot-product attention.

Problem: B=16, Tq=Tk=D=1024, fp32.
    scores = Q @ E^T ; weights = softmax(scores, -1) ; out = weights @ E

Sharding: data-parallel over batch — each of the 8 NeuronCores processes
2 batches end-to-end; no cross-core communication.

Layout strategy: the host-side sharding step (inside kernel()) rearranges
each core's inputs so no on-device transposition of Q or E is needed:
  - q is shipped per q-block as [qb, d-part, dc, j] (i.e. Q^T tiled), so
    each 128-row q-block's stationary operands DMA straight into SBUF.
  - e is shipped twice: natural [k-part, kc, d] (bmm2 rhs) and transposed
    [d-part, dc, k] (bmm1 rhs). One 2 MB DMA each per batch.
q/et are shipped as fp16: the 11-bit mantissa keeps the softmax-amplified
score error ~6e-3. fp8 variants measured over the 2e-2 gate (bmm2 with
e4m3 W and E alone gives 2.7e-2 — softmax rows here are near-one-hot and
near-tie weights quantize at ~3%), so both bmms stay on the 1-cycle/row
16-bit path. The output is stored fp16 and widened to fp32 on the host.
Measured rel_l2 = 2.289e-3 vs the fp32 reference (gate 2e-2).

The PE matmul stream is the wall: walrus emits one LDWEIGHTS per matmul
(512 per rep; --enable-ldw-opt is hardcoded off and crashes walrus when
forced on), but an mm-only ablation measures ~110-115 us/core ~= the
512 x 213 ns pure-streaming roofline, i.e. LDWEIGHTS is effectively
hidden by the PE's load-ahead. So this revision removes everything else
from the PE stream and every stall around it:
  - W^T for bmm2 goes through the DMA crossbar transpose (16x128 xbar
    tiles, ~0.9 us/block on the SDMA engines) instead of PE
    transpose-mode — no transposes or identity (re)loads on the PE.
    (N=1024 moving operands would halve instruction count but are
    rejected by the ISA: fp32 PSUM output would span 2 banks.)
  - The depth-3 software pipeline is FLAT across blocks, batches, and
    in-NEFF reps: PE order is bmm1(i+2), bmm2(i) over the whole
    sequence, so each block's max->exp->xbar-transpose chain (~4 us)
    has two bmm1 + two bmm2 slots (~14 us) of cover and the pipeline
    never drains at batch boundaries.
PSUM: 2x scores + 2x ctx [128,1024] fp32 tiles = all 8 banks.

Per-block stages:
  front: DMA q-block tiles, bmm1 into PSUM dc-outer.
  mid:   negated row-max (DVE) -> exp with per-partition bias and fused
         row-sum (ACT, bf16 out) -> reciprocal (DVE) -> W^T via
         dma_start_transpose (sync/HWDGE queue).
  back:  bmm2 kc-outer -> fold 1/rowsum into the PSUM->SBUF copy (DVE,
         fp16 out) -> DMA out.

One scheduling detail worth its own note: reciprocal(i+1) waits on
exp(i+1)'s row-sum, so it must be emitted AFTER the block-i output
scale in the DVE FIFO or it head-of-line blocks it (measured ~4 us).

Measured (paired p10-estimator, R=32 differencing): ~110 us/core vs
~114 us/core for the unreordered variant and ~131 us/core for the
PE-transpose per-batch-pipeline predecessor in paired sessions; the
mm-only ablation measures the same ~110 us, i.e. the kernel runs at
the matmul stream's own pace: 512 MMs x 215.6 ns vs the 213.3 ns/MM
N=512 fp16 streaming roofline (~99%).
"""

import numpy as np

import concourse.bass as bass
import concourse.tile as tile
from concourse import bacc, mybir

P = 128
B_PER_CORE = 2
T = 1024  # Tq = Tk
D = 1024
NC_CHUNKS = T // P  # 8 k-chunks / q-blocks
ND_CHUNKS = D // P  # 8 d-chunks
F32 = mybir.dt.float32
BF16 = mybir.dt.bfloat16
F16 = mybir.dt.float16


def build_nc(reps: int = 1, mm_only: bool = False, salt: float | None = None):
    nc = bacc.Bacc("TRN2", target_bir_lowering=False, debug=False)
    # q: [b, qb, p, dc*128+j] = Q[b, qb*128+j, dc*128+p]  (Q^T, block-tiled)
    q_dram = nc.dram_tensor(
        "q", [B_PER_CORE, NC_CHUNKS, P, D], F16, kind="ExternalInput"
    ).ap()
    # e: [b, p, kc*1024+d] = E[b, kc*128+p, d]  (natural, partition-tiled)
    e_dram = nc.dram_tensor(
        "e", [B_PER_CORE, P, NC_CHUNKS * D], BF16, kind="ExternalInput"
    ).ap()
    # et: [b, p, dc*1024+k] = E[b, k, dc*128+p]  (transposed, partition-tiled)
    et_dram = nc.dram_tensor(
        "et", [B_PER_CORE, P, ND_CHUNKS * T], F16, kind="ExternalInput"
    ).ap()
    o_dram = nc.dram_tensor("o", [B_PER_CORE, T, D], F16, kind="ExternalOutput").ap()

    with tile.TileContext(nc) as tc:
        with (
            tc.tile_pool(name="const", bufs=1) as const_pool,
            tc.tile_pool(name="e_r", bufs=2) as e_r_pool,
            tc.tile_pool(name="etr", bufs=2) as etr_pool,
            tc.tile_pool(name="qt", bufs=3) as qt_pool,
            tc.tile_pool(name="w", bufs=3) as w_pool,
            tc.tile_pool(name="wt", bufs=3) as wt_pool,
            tc.tile_pool(name="ctx", bufs=2) as ctx_pool,
            tc.tile_pool(name="stat", bufs=4) as stat_pool,
            tc.tile_pool(name="sc_ps", bufs=2, space="PSUM") as sc_psum,
            tc.tile_pool(name="ctx_ps", bufs=2, space="PSUM") as ctx_psum,
        ):
            wt_const = None
            if mm_only:
                wt_const = const_pool.tile([P, NC_CHUNKS, P], BF16)
                nc.vector.memset(wt_const[:], 0.001)
            if salt is not None:
                # distinct BIR/HLO so compile caches can't cross-contaminate
                # otherwise-identical variants
                salt_t = const_pool.tile([P, 1], F32)
                nc.vector.memset(salt_t[:], salt)

            # flat block sequence across reps and batches
            batches_seq = [b for _ in range(reps) for b in range(B_PER_CORE)]
            blocks = [
                (bi, qb) for bi in range(len(batches_seq)) for qb in range(NC_CHUNKS)
            ]
            n_blocks = len(blocks)
            e_tiles = {}

            def ensure_batch(bi):
                if bi in e_tiles or bi >= len(batches_seq):
                    return
                b = batches_seq[bi]
                e_r = e_r_pool.tile([P, NC_CHUNKS, D], BF16, name="e_r")
                nc.gpsimd.dma_start(e_r[:], e_dram[b])
                etr = etr_pool.tile([P, ND_CHUNKS, T], F16, name="etr")
                nc.gpsimd.dma_start(etr[:], et_dram[b])
                e_tiles[bi] = (e_r, etr)

            def emit_front(i):
                """DMA Q^T block, bmm1 into PSUM (dc outer: matmul pairs
                share the stationary operand)."""
                bi, qb = blocks[i]
                ensure_batch(bi)
                etr = e_tiles[bi][1]
                b = batches_seq[bi]
                qt = qt_pool.tile([P, ND_CHUNKS, P], F16, name="qt")
                nc.sync.dma_start(qt[:], q_dram[b, qb])
                sc_ps = sc_psum.tile([P, T], F32, name="sc_ps")
                for dc in range(ND_CHUNKS):
                    for kh in range(2):
                        nc.tensor.matmul(
                            sc_ps[:, kh * 512 : (kh + 1) * 512],
                            qt[:, dc, :],
                            etr[:, dc, kh * 512 : (kh + 1) * 512],
                            start=(dc == 0),
                            stop=(dc == ND_CHUNKS - 1),
                        )
                return sc_ps

            def emit_mid(i, sc_ps):
                """Softmax the block's scores; W^T via the DMA crossbar."""
                if mm_only:
                    return None
                negmax = stat_pool.tile([P, 1], F32, tag="negmax", name="negmax")
                nc.vector.tensor_reduce(
                    out=negmax[:],
                    in_=sc_ps[:],
                    op=mybir.AluOpType.max,
                    axis=mybir.AxisListType.X,
                    negate=True,
                )
                w_sb = w_pool.tile([P, T], BF16, name="w_sb")
                ssum = stat_pool.tile([P, 1], F32, tag="ssum", name="ssum")
                nc.scalar.activation(
                    w_sb[:],
                    sc_ps[:],
                    mybir.ActivationFunctionType.Exp,
                    bias=negmax[:],
                    accum_out=ssum[:],
                )
                recip = stat_pool.tile([P, 1], F32, tag="recip", name="recip")
                nc.vector.reciprocal(recip[:], ssum[:])
                wt = wt_pool.tile([P, NC_CHUNKS, P], BF16, name="wt")
                nc.sync.dma_start_transpose(wt[:], w_sb[:])
                return wt, recip

            def emit_back(i, mid, sc_ps=None):
                """bmm2 kc-outer; fold 1/rowsum into the PSUM->SBUF copy;
                DMA out."""
                bi, qb = blocks[i]
                e_r = e_tiles[bi][0]
                b = batches_seq[bi]
                if mm_only:
                    wt, recip = wt_const, None
                else:
                    wt, recip = mid
                ctx_ps = ctx_psum.tile([P, T], F32, name="ctx_ps")
                for kc in range(NC_CHUNKS):
                    for dh in range(2):
                        nc.tensor.matmul(
                            ctx_ps[:, dh * 512 : (dh + 1) * 512],
                            wt[:, kc, :],
                            e_r[:, kc, dh * 512 : (dh + 1) * 512],
                            start=(kc == 0),
                            stop=(kc == NC_CHUNKS - 1),
                        )
                ctx_sb = ctx_pool.tile([P, D], F16, name="ctx_sb")
                if mm_only:
                    nc.vector.tensor_copy(ctx_sb[:], ctx_ps[:])
                    nc.vector.tensor_copy(ctx_sb[:, 0:1], sc_ps[:, 0:1])
                else:
                    nc.vector.tensor_scalar_mul(ctx_sb[:], ctx_ps[:], recip[:])
                nc.sync.dma_start(o_dram[b, qb * P : (qb + 1) * P, :], ctx_sb[:])

            # depth-3 pipeline over the flat block list; PE order:
            # f0 f1 f2 b0 f3 b1 ...  back(i) is emitted BEFORE mid(i+1):
            # recip(i+1) waits on exp(i+1), and emitting it after scale(i)
            # keeps it from blocking scale(i) in the DVE FIFO (measured
            # ~4 us/rep).
            fronts = {0: emit_front(0), 1: emit_front(1)}
            mids = {0: emit_mid(0, fronts[0])}
            for i in range(n_blocks):
                if i + 2 < n_blocks:
                    fronts[i + 2] = emit_front(i + 2)
                emit_back(i, mids.get(i), sc_ps=fronts[i])
                if i + 1 < n_blocks:
                    mids[i + 1] = emit_mid(i + 1, fronts[i + 1])
                fronts.pop(i, None)
                mids.pop(i, None)

    nc.compile()
    return nc


def make_in_maps(decoder_hidden: np.ndarray, encoder_outputs: np.ndarray):
    """Host-side sharding + layout prep: per-core input dicts matching the
    DRAM tensor layouts declared in build_nc."""
    dh = np.asarray(decoder_hidden, dtype=np.float32)
    eo = np.asarray(encoder_outputs, dtype=np.float32)
    assert dh.shape == (16, T, D) and eo.shape == (16, T, D)
    import ml_dtypes

    in_maps = []
    for i in range(8):
        qc = dh[i * B_PER_CORE : (i + 1) * B_PER_CORE]
        ec = eo[i * B_PER_CORE : (i + 1) * B_PER_CORE]
        # [b, qb, j, dc, p] -> [b, qb, p, dc, j], fp16 (bmm1 stationary)
        qh = np.ascontiguousarray(
            qc.reshape(B_PER_CORE, NC_CHUNKS, P, ND_CHUNKS, P)
            .transpose(0, 1, 4, 3, 2)
            .astype(np.float16)
        ).reshape(B_PER_CORE, NC_CHUNKS, P, D)
        # [b, kc, p, d] -> [b, p, kc, d], bf16 for bmm2's rhs
        eh = np.ascontiguousarray(
            ec.reshape(B_PER_CORE, NC_CHUNKS, P, D)
            .transpose(0, 2, 1, 3)
            .astype(ml_dtypes.bfloat16)
        ).reshape(B_PER_CORE, P, NC_CHUNKS * D)
        # [b, k, dc, p] -> [b, p, dc, k], fp16 (bmm1 moving operand)
        eth = np.ascontiguousarray(
            ec.reshape(B_PER_CORE, T, ND_CHUNKS, P)
            .transpose(0, 3, 2, 1)
            .astype(np.float16)
        ).reshape(B_PER_CORE, P, ND_CHUNKS * T)
        in_maps.append({"q": qh, "e": eh, "et": eth})
    return in_maps


_NC_CACHE = None


def _get_nc():
    global _NC_CACHE
    if _NC_CACHE is None:
        _NC_CACHE = build_nc()
    return _NC_CACHE


def kernel(decoder_hidden: np.ndarray, encoder_outputs: np.ndarray) -> np.ndarray:
    import os

    # The axon client here has no NTFF profiling hook; make sure a stray
    # BASS_TRACE in the environment can't push run_bass_kernel_spmd onto
    # the tracing path.
    os.environ["BASS_NEVER_TRACE"] = "1"
    from concourse import bass_utils

    nc = _get_nc()
    in_maps = make_in_maps(decoder_hidden, encoder_outputs)
    res = bass_utils.run_bass_kernel_spmd(nc, in_maps, core_ids=list(range(8)))
    return np.concatenate([r["o"] for r in res.results], axis=0).astype(np.float32)
